# revision 18
# baseline (speedup 1.0000x reference)
"""Trainium2 Bass kernel for nn_MessagePassingLayer (GNN message passing).

reference semantics (per batch b):
  cm  = adj[b].T @ ps[b]                  # [C, H] channel aggregation
  ncs = GRUCell(x=cs[b], h=cm)            # new channel states
  pm  = adj[b] @ ncs                      # [P, H] path aggregation
  nps = GRUCell(x=ps[b], h=pm)            # new path states
  returns (nps, ncs)

Sharding: data-parallel over batch, 2 batches per core x 8 cores.

Per-core design (memory-regime: adj is 16MB/batch and is the traffic):
  - adj[b] is DMA'd from HBM ONCE (f32->bf16 cast in flight, SWDGE),
    consumed as p-slabs [128, C] by einsum1 (moving operand), and
    PE-transposed tile-by-tile into a persistent AT [c_lo, j, p] bf16
    for einsum2. The reference reads adj twice; we read once.
  - einsum1 computed transposed: cmT[h, c] += ps_tile.T-form matmuls
    (lhsT = ps tile [p,H], rhs = A slab) accumulating in PSUM f32.
  - GRU gates feature-major: giT/ghT [3H, n] = w^T-form matmuls with
    rhs = xT/hT [H, n]; biases are per-partition ACT bias APs.
  - einsum2 transposed: pmT[h, p] += (lhsT = ncs tile [c,H],
    rhs = AT slab [c, p]) accumulating in PSUM f32.
  - outputs packed on-chip to [q, (g l h)] so each partition's HBM run
    is 512B (DMA line-rate), via stride-4 PE transposes.
"""

import numpy as np

import concourse.bass as bass
import concourse.tile as tile
from concourse import bacc, masks, mybir
from concourse.bass_utils import run_bass_kernel_spmd

F32 = mybir.dt.float32
# 2-byte compute dtype: fp16 (10-bit mantissa) — adj in [0,1), states O(1),
# messages O(30): all comfortably in fp16 range, 4x less rounding than bf16.
BF16 = mybir.dt.float16

B, P, C, H = 16, 2048, 2048, 32
G = 3 * H  # 96
NCORES = 8
BPC = B // NCORES  # batches per core
PB = 128  # partition block
NP = P // PB  # 16 p-chunks
NC = C // PB  # 16 c-chunks
NKC = 512  # matmul moving chunk (one PSUM f32 bank)


def _gru_elementwise(nc, pool, gates, hT, b_rz, b_hhn, b_ihn, out, dt):
    """Feature-major GRU combine for one column block.

    gates: SBUF tile [G, 2, F]; [:, 0, :] = giT = w_ih @ xT,
    [:, 1, :] = ghT = w_hh @ hT (both WITHOUT biases).
    hT: SBUF [H, F] slice (the hidden/message state).
    out <- (1-z)*n + z*h.
    """
    F = gates.shape[-1]
    gi = gates[:, 0, :]
    gh = gates[:, 1, :]

    # All TensorTensor inputs must share base partition (walrus
    # constraint) -> every gate slice is routed through a single-input
    # ACT op landing at base partition 0 (which also folds its bias).
    # 5 rotating temp slots; no two simultaneously-live tiles share a tag.
    AF = mybir.ActivationFunctionType
    s_rz = pool.tile([2 * H, F], dt, tag="gru_t1")
    nc.vector.tensor_add(s_rz[:], gi[0 : 2 * H, :], gh[0 : 2 * H, :])
    r = pool.tile([H, F], dt, tag="gru_t2")
    nc.scalar.activation(r[:], s_rz[0:H, :], AF.Sigmoid, bias=b_rz[0:H, :])
    z = pool.tile([H, F], dt, tag="gru_t3")
    nc.scalar.activation(z[:], s_rz[H : 2 * H, :], AF.Sigmoid,
                         bias=b_rz[H : 2 * H, :])
    g = pool.tile([H, F], dt, tag="gru_t4")
    nc.scalar.activation(g[:], gh[2 * H : G, :], AF.Identity, bias=b_hhn)
    gin = pool.tile([H, F], dt, tag="gru_t1")
    nc.scalar.activation(gin[:], gi[2 * H : G, :], AF.Identity, bias=b_ihn)
    t1 = pool.tile([H, F], dt, tag="gru_t5")
    nc.vector.tensor_mul(t1[:], r[:], g[:])
    npre = pool.tile([H, F], dt, tag="gru_t4")
    nc.vector.tensor_add(npre[:], gin[:], t1[:])
    ng = pool.tile([H, F], dt, tag="gru_t5")
    nc.scalar.activation(ng[:], npre[:], AF.Tanh)
    d = pool.tile([H, F], dt, tag="gru_t1")
    nc.vector.tensor_sub(d[:], hT, ng[:])
    zd = pool.tile([H, F], dt, tag="gru_t4")
    nc.vector.tensor_mul(zd[:], z[:], d[:])
    nc.vector.tensor_add(out[:], ng[:], zd[:])
    return out


def _gru(tc, gru_pool, ps_gate, wT_ih, wT_hh, xT, hT, b_rz, b_ih, b_hh,
         st_pool, out_tag, dt=BF16, nblocks=1):
    """Full feature-major GRUCell: returns SBUF [H, N] tile (dtype dt).

    Gate matmuls in [G, 2, 512] f32 PSUM quarters (2 banks), ACT-evac'd
    to SBUF [G, 2, FW] blocks, then per-block elementwise. The path GRU
    uses dt=f32 + nblocks=2: its hidden (path_msg) reaches ~1e5, far
    outside fp16 range, and the z-gate there multiplies those huge
    values, so the whole h-side chain must stay f32.
    """
    nc = tc.nc
    N = xT.shape[-1]
    out = st_pool.tile([H, N], dt, tag=out_tag)
    FW = N // nblocks
    for blk in range(nblocks):
        gates = gru_pool.tile([G, 2, FW], dt, tag="gates")
        for q in range(FW // NKC):
            gp = ps_gate.tile([G, 2, NKC], F32, tag="gate")
            sl = slice(blk * FW + q * NKC, blk * FW + (q + 1) * NKC)
            qs = slice(q * NKC, (q + 1) * NKC)
            nc.tensor.matmul(gp[:, 0, :], wT_ih[:], xT[:, sl],
                             start=True, stop=True)
            nc.tensor.matmul(gp[:, 1, :], wT_hh[:], hT[:, sl],
                             start=True, stop=True)
            nc.scalar.copy(gates[:, :, qs], gp[:])
        bsl = slice(blk * FW, (blk + 1) * FW)
        _gru_elementwise(nc, gru_pool, gates, hT[:, bsl], b_rz[:],
                         b_hh[2 * H : G, :], b_ih[2 * H : G, :],
                         out[:, bsl], dt)
    return out


def build_nc(debug_outputs=False):
    nc = bacc.Bacc("TRN2", target_bir_lowering=False, debug=False,
                   num_devices=NCORES)

    adj = nc.dram_tensor("adj", [BPC, P, C], F32, kind="ExternalInput")
    ps = nc.dram_tensor("ps", [BPC, P, H], F32, kind="ExternalInput")
    cs = nc.dram_tensor("cs", [BPC, C, H], F32, kind="ExternalInput")
    w_ih_c = nc.dram_tensor("w_ih_c", [G, H], F32, kind="ExternalInput")
    w_hh_c = nc.dram_tensor("w_hh_c", [G, H], F32, kind="ExternalInput")
    w_ih_p = nc.dram_tensor("w_ih_p", [G, H], F32, kind="ExternalInput")
    w_hh_p = nc.dram_tensor("w_hh_p", [G, H], F32, kind="ExternalInput")
    b_ih_c = nc.dram_tensor("b_ih_c", [G, 1], F32, kind="ExternalInput")
    b_hh_c = nc.dram_tensor("b_hh_c", [G, 1], F32, kind="ExternalInput")
    b_ih_p = nc.dram_tensor("b_ih_p", [G, 1], F32, kind="ExternalInput")
    b_hh_p = nc.dram_tensor("b_hh_p", [G, 1], F32, kind="ExternalInput")
    out_np = nc.dram_tensor("new_path", [BPC, P, H], F32, kind="ExternalOutput")
    out_nc = nc.dram_tensor("new_channel", [BPC, C, H], F32, kind="ExternalOutput")
    dbg = {}
    if debug_outputs:
        dbg["cmT"] = nc.dram_tensor("dbg_cmT", [BPC, H, C], F32, kind="ExternalOutput")
        dbg["pmT"] = nc.dram_tensor("dbg_pmT", [BPC, H, P], F32, kind="ExternalOutput")
        dbg["ncsT"] = nc.dram_tensor("dbg_ncsT", [BPC, H, C], F32, kind="ExternalOutput")

    with tile.TileContext(nc) as tc:
        _body(tc, adj, ps, cs,
              (w_ih_c, w_hh_c, b_ih_c, b_hh_c),
              (w_ih_p, w_hh_p, b_ih_p, b_hh_p),
              out_np, out_nc, dbg)
    nc.finalize()
    return nc


def _body(tc, adj, ps, cs, wc, wp, out_np, out_nc, dbg):
    nc = tc.nc
    from contextlib import ExitStack

    ctx = ExitStack()
    with ctx:
        const = ctx.enter_context(tc.tile_pool(name="const", bufs=1))
        a_pool = ctx.enter_context(tc.tile_pool(name="a_slabs", bufs=4))
        at_pool = ctx.enter_context(tc.tile_pool(name="at", bufs=2))
        st_pool = ctx.enter_context(tc.tile_pool(name="states", bufs=1))
        gru_pool = ctx.enter_context(tc.tile_pool(name="gru", bufs=1))
        out_pool = ctx.enter_context(tc.tile_pool(name="outs", bufs=2))
        # PSUM: 4 + 2 + 1 + 1 = 8 banks
        ps_mm = ctx.enter_context(tc.tile_pool(name="ps_mm", bufs=1, space="PSUM"))
        ps_gate = ctx.enter_context(tc.tile_pool(name="ps_gate", bufs=1, space="PSUM"))
        ps_tp = ctx.enter_context(tc.tile_pool(name="ps_tp", bufs=1, space="PSUM"))
        ps_sm = ctx.enter_context(tc.tile_pool(name="ps_sm", bufs=1, space="PSUM"))

        ident = const.tile([PB, PB], BF16)
        masks.make_identity(nc, ident[:])
        ident_f = const.tile([PB, PB], F32)
        masks.make_identity(nc, ident_f[:])
        idents = {BF16: ident, F32: ident_f}

        # ---- weights: load [G, H], PE-transpose to [H, G] ----
        # hhp stays f32: it multiplies path_msg (~1e5 scale) where the
        # z-gate argument needs small absolute error.
        wT = {}
        for name, wdram, wdt in (("ihc", wc[0], BF16), ("hhc", wc[1], BF16),
                                 ("ihp", wp[0], BF16), ("hhp", wp[1], F32)):
            w_ld = const.tile([G, H], wdt, tag=f"w_{name}")
            nc.gpsimd.dma_start(w_ld[:], wdram[:, :])
            wt_ps = ps_sm.tile([H, G], wdt, tag="sm")
            nc.tensor.transpose(wt_ps[:], w_ld[:], idents[wdt][0:G, 0:G])
            wt = const.tile([H, G], wdt, tag=f"wT_{name}")
            nc.scalar.copy(wt[:], wt_ps[:])
            wT[name] = wt

        # ---- biases ----
        bias = {}
        for name, bdram in (("ihc", wc[2]), ("hhc", wc[3]),
                            ("ihp", wp[2]), ("hhp", wp[3])):
            bt = const.tile([G, 1], F32, tag=f"b_{name}")
            nc.sync.dma_start(bt[:], bdram[:, :])
            bias[name] = bt
        b_rz = {}
        for s in ("c", "p"):
            t = const.tile([2 * H, 1], F32, tag=f"brz_{s}")
            nc.vector.tensor_add(t[:], bias["ih" + s][0 : 2 * H, :],
                                 bias["hh" + s][0 : 2 * H, :])
            b_rz[s] = t

        for b in range(BPC):
            # ---- states: natural tiles (cast-DMA) + feature-major via PE ----
            ps_nat = st_pool.tile([PB, NP, H], BF16, tag="ps_nat")
            nc.gpsimd.dma_start(
                ps_nat[:], ps[b].rearrange("(i p) h -> p i h", p=PB))
            cs_nat = st_pool.tile([PB, NC, H], BF16, tag="cs_nat")
            nc.gpsimd.dma_start(
                cs_nat[:], cs[b].rearrange("(i p) h -> p i h", p=PB))

            sT = {}
            for nm, nat, nch in (("psT", ps_nat, NP), ("csT", cs_nat, NC)):
                dst = st_pool.tile([H, nch * PB], BF16, tag=nm)
                for half in range(2):
                    tp = ps_sm.tile([H, nch // 2, PB], BF16, tag="sm")
                    for i in range(nch // 2):
                        ii = half * (nch // 2) + i
                        nc.tensor.transpose(tp[:, i, :], nat[:, ii, :],
                                            ident[0:PB, 0:PB])
                    nc.scalar.copy(
                        dst[:, half * (nch // 2) * PB : (half + 1) * (nch // 2) * PB],
                        tp[:],
                    )
                sT[nm] = dst

            # ---- stream A: einsum1 (cmT) + transposes into AT ----
            at = at_pool.tile([PB, NC, P], BF16, tag="at")
            cmT = ps_mm.tile([H, C], F32, tag="mm")
            for i in range(NP):
                slab = a_pool.tile([PB, C], BF16, tag="a")
                nc.gpsimd.dma_start(slab[:], adj[b, i * PB : (i + 1) * PB, :])
                for n in range(C // NKC):
                    nc.tensor.matmul(
                        cmT[:, n * NKC : (n + 1) * NKC],
                        ps_nat[:, i, :],
                        slab[:, n * NKC : (n + 1) * NKC],
                        start=(i == 0), stop=(i == NP - 1),
                    )
                for half in range(2):
                    tp = ps_tp.tile([PB, NC // 2, PB], BF16, tag="tp")
                    for jj in range(NC // 2):
                        j = half * (NC // 2) + jj
                        nc.tensor.transpose(
                            tp[:, jj, :], slab[:, j * PB : (j + 1) * PB],
                            ident[:, :])
                    nc.scalar.copy(
                        at[:, half * (NC // 2) : (half + 1) * (NC // 2),
                           i * PB : (i + 1) * PB],
                        tp[:],
                    )

            # ---- GRU-c ----
            cmT_s = st_pool.tile([H, C], BF16, tag="cmT_s")
            nc.scalar.copy(cmT_s[:], cmT[:])
            if "cmT" in dbg:
                nc.gpsimd.dma_start(dbg["cmT"][b], cmT_s[:])

            ncsT = _gru(tc, gru_pool, ps_gate, wT["ihc"], wT["hhc"],
                        sT["csT"], cmT_s, b_rz["c"], bias["ihc"], bias["hhc"],
                        st_pool, "ncsT", dt=BF16, nblocks=2)

            if "ncsT" in dbg:
                nc.gpsimd.dma_start(dbg["ncsT"][b], ncsT[:])

            # ncs natural tiles [c_lo, j, H] for einsum2 lhsT
            ncs_nat = st_pool.tile([PB, NC, H], BF16, tag="ncs_nat")
            tpn = ps_sm.tile([PB, NC, H], BF16, tag="sm")
            for j in range(NC):
                nc.tensor.transpose(tpn[:, j, :], ncsT[:, j * PB : (j + 1) * PB],
                                    ident[0:H, 0:H])
            nc.vector.tensor_copy(ncs_nat[:], tpn[:])

            # packed output new_channel
            _pack_out(tc, ncsT, out_nc[b], ps_sm, out_pool, idents)

            # ---- einsum2: pmT ----
            pmT = ps_mm.tile([H, P], F32, tag="mm")
            for j in range(NC):
                for n in range(P // NKC):
                    nc.tensor.matmul(
                        pmT[:, n * NKC : (n + 1) * NKC],
                        ncs_nat[:, j, :],
                        at[:, j, n * NKC : (n + 1) * NKC],
                        start=(j == 0), stop=(j == NC - 1),
                    )
            pmT_s = st_pool.tile([H, P], F32, tag="pmT_s")
            nc.scalar.copy(pmT_s[:], pmT[:])
            if "pmT" in dbg:
                nc.sync.dma_start(dbg["pmT"][b], pmT_s[:])

            # ---- GRU-p ----
            npT = _gru(tc, gru_pool, ps_gate, wT["ihp"], wT["hhp"],
                       sT["psT"], pmT_s, b_rz["p"], bias["ihp"], bias["hhp"],
                       st_pool, "npT", dt=F32, nblocks=4)

            _pack_out(tc, npT, out_np[b], ps_sm, out_pool, idents)


def _pack_out(tc, srcT, dram_b, ps_sm, out_pool, idents):
    """srcT [H, N] -> HBM [N, H] f32 with 512B-per-partition runs.

    Packed SBUF layout [q, g, l, h]: row index n = 512*g + 4*q + l.
    """
    nc = tc.nc
    dt = srcT.dtype
    N = srcT.shape[-1]
    NG = N // 512
    src_r = srcT.rearrange("h (g q l) -> h g q l", g=NG, l=4)
    pk = ps_sm.tile([PB, NG, 4, H], dt, tag="sm")
    for g in range(NG):
        for l in range(4):
            nc.tensor.transpose(pk[:, g, l, :], src_r[:, g, :, l],
                                idents[dt][0:H, 0:H])
    sb = out_pool.tile([PB, NG, 4, H], F32, tag="opack")
    nc.scalar.copy(sb[:], pk[:])
    nc.sync.dma_start(
        dram_b.rearrange("(g q l) h -> q g l h", q=PB, l=4), sb[:])


# ---------------------------------------------------------------------------
# host-side entry
# ---------------------------------------------------------------------------

_NC_CACHE = {}


def _get_nc(debug_outputs=False):
    key = bool(debug_outputs)
    if key not in _NC_CACHE:
        _NC_CACHE[key] = build_nc(debug_outputs=key)
    return _NC_CACHE[key]


def kernel(path_states, channel_states, adj_matrix,
           w_ih_c, w_hh_c, b_ih_c, b_hh_c,
           w_ih_p, w_hh_p, b_ih_p, b_hh_p,
           _debug=False, _trace=False):
    nc = _get_nc(debug_outputs=_debug)
    f32 = np.float32
    in_maps = []
    for k in range(NCORES):
        s = slice(k * BPC, (k + 1) * BPC)
        in_maps.append({
            "adj": np.ascontiguousarray(adj_matrix[s], f32),
            "ps": np.ascontiguousarray(path_states[s], f32),
            "cs": np.ascontiguousarray(channel_states[s], f32),
            "w_ih_c": np.ascontiguousarray(w_ih_c, f32),
            "w_hh_c": np.ascontiguousarray(w_hh_c, f32),
            "w_ih_p": np.ascontiguousarray(w_ih_p, f32),
            "w_hh_p": np.ascontiguousarray(w_hh_p, f32),
            "b_ih_c": np.ascontiguousarray(b_ih_c, f32).reshape(G, 1),
            "b_hh_c": np.ascontiguousarray(b_hh_c, f32).reshape(G, 1),
            "b_ih_p": np.ascontiguousarray(b_ih_p, f32).reshape(G, 1),
            "b_hh_p": np.ascontiguousarray(b_hh_p, f32).reshape(G, 1),
        })
    res = run_bass_kernel_spmd(nc, in_maps, core_ids=list(range(NCORES)),
                               trace=_trace)
    new_path = np.concatenate([res.results[k]["new_path"] for k in range(NCORES)])
    new_channel = np.concatenate(
        [res.results[k]["new_channel"] for k in range(NCORES)])
    out = (new_path, new_channel)
    if _debug or _trace:
        return out, res
    return out


# revision 22
# speedup vs baseline: 1.0549x; 1.0549x over previous
"""Trainium2 Bass kernel for nn_MessagePassingLayer (GNN message passing).

reference semantics (per batch b):
  cm  = adj[b].T @ ps[b]                  # [C, H] channel aggregation
  ncs = GRUCell(x=cs[b], h=cm)            # new channel states
  pm  = adj[b] @ ncs                      # [P, H] path aggregation
  nps = GRUCell(x=ps[b], h=pm)            # new path states
  returns (nps, ncs)

Sharding: data-parallel over batch, 2 batches per core x 8 cores.

Per-core design (memory-regime: adj is 16MB/batch and is the traffic):
  - adj[b] is DMA'd from HBM ONCE (f32->bf16 cast in flight, SWDGE),
    consumed as p-slabs [128, C] by einsum1 (moving operand), and
    PE-transposed tile-by-tile into a persistent AT [c_lo, j, p] bf16
    for einsum2. The reference reads adj twice; we read once.
  - einsum1 computed transposed: cmT[h, c] += ps_tile.T-form matmuls
    (lhsT = ps tile [p,H], rhs = A slab) accumulating in PSUM f32.
  - GRU gates feature-major: giT/ghT [3H, n] = w^T-form matmuls with
    rhs = xT/hT [H, n]; biases are per-partition ACT bias APs.
  - einsum2 transposed: pmT[h, p] += (lhsT = ncs tile [c,H],
    rhs = AT slab [c, p]) accumulating in PSUM f32.
  - outputs packed on-chip to [q, (g l h)] so each partition's HBM run
    is 512B (DMA line-rate), via stride-4 PE transposes.
"""

import numpy as np

import concourse.bass as bass
import concourse.tile as tile
from concourse import bacc, masks, mybir
from concourse.bass_utils import run_bass_kernel_spmd

F32 = mybir.dt.float32
# 2-byte compute dtype: fp16 (10-bit mantissa) — adj in [0,1), states O(1),
# messages O(30): all comfortably in fp16 range, 4x less rounding than bf16.
BF16 = mybir.dt.float16

B, P, C, H = 16, 2048, 2048, 32
G = 3 * H  # 96
NCORES = 8
BPC = B // NCORES  # batches per core
PB = 128  # partition block
NP = P // PB  # 16 p-chunks
NC = C // PB  # 16 c-chunks
NKC = 512  # matmul moving chunk (one PSUM f32 bank)


def _gru(tc, pool, ps_misc, wT_ih, wT_hh, xT, hT, b_rz, bias_n,
         st_pool, out_tag, dt_b=BF16, g_engine="act"):
    """Feature-major GRUCell -> SBUF [H, N] tile (dtype dt).

    Per 512-col chunk, one PSUM tile [128, 512] f32 holds:
      rows 0:64   = i_rz + h_rz   (two accumulating matmuls)
      rows 64:96  = i_n
      rows 96:128 = h_n
    r/z/gin/g are produced by single-input ACT/DVE ops reading PSUM
    (bias folded, base partition moved to 0), then 5 full-width
    TensorTensor ops combine. No gates SBUF tile, no separate adds.
    """
    nc = tc.nc
    AF = mybir.ActivationFunctionType
    N = xT.shape[-1]
    out = st_pool.tile([H, N], dt_b, tag=out_tag)
    r = pool.tile([H, N], BF16, tag="gru_r")
    z = pool.tile([H, N], BF16, tag="gru_z")
    g = pool.tile([H, N], BF16, tag="gru_g")
    gin = pool.tile([H, N], BF16, tag="gru_gin")
    for q in range(N // NKC):
        gp = ps_misc.tile([PB, NKC], F32, tag="sm")
        sl = slice(q * NKC, (q + 1) * NKC)
        nc.tensor.matmul(gp[0 : 2 * H, :], wT_ih[:, 0 : 2 * H], xT[:, sl],
                         start=True, stop=False)
        nc.tensor.matmul(gp[0 : 2 * H, :], wT_hh[:, 0 : 2 * H], hT[:, sl],
                         start=False, stop=True)
        nc.tensor.matmul(gp[2 * H : G, :], wT_ih[:, 2 * H : G], xT[:, sl],
                         start=True, stop=True)
        nc.tensor.matmul(gp[G : G + H, :], wT_hh[:, 2 * H : G], hT[:, sl],
                         start=True, stop=True, tile_position=(0, 96))
        nc.scalar.activation(r[:, sl], gp[0:H, :], AF.Sigmoid, bias=b_rz[0:H, :])
        nc.scalar.activation(z[:, sl], gp[H : 2 * H, :], AF.Sigmoid,
                             bias=b_rz[H : 2 * H, :])
        if g_engine == "act":
            nc.scalar.activation(g[:, sl], gp[G : G + H, :], AF.Identity,
                                 bias=bias_n[G : G + H, :])
        else:
            nc.vector.tensor_scalar_add(g[:, sl], gp[G : G + H, :],
                                        bias_n[G : G + H, :])
        nc.vector.tensor_scalar_add(gin[:, sl], gp[2 * H : G, :],
                                    bias_n[2 * H : G, :])
    t1 = pool.tile([H, N], BF16, tag="gru_t1")
    nc.vector.tensor_mul(t1[:], r[:], g[:])
    npre = pool.tile([H, N], BF16, tag="gru_g")
    nc.vector.tensor_add(npre[:], gin[:], t1[:])
    ng = pool.tile([H, N], dt_b, tag="gru_t1")
    nc.scalar.activation(ng[:], npre[:], AF.Tanh)
    d = pool.tile([H, N], dt_b, tag="gru_g")
    nc.vector.tensor_sub(d[:], hT, ng[:])
    zd = pool.tile([H, N], dt_b, tag="gru_gin")
    nc.vector.tensor_mul(zd[:], z[:], d[:])
    nc.vector.tensor_add(out[:], ng[:], zd[:])
    return out


def build_nc(debug_outputs=False):
    nc = bacc.Bacc("TRN2", target_bir_lowering=False, debug=False,
                   num_devices=NCORES)

    adj = nc.dram_tensor("adj", [BPC, P, C], F32, kind="ExternalInput")
    ps = nc.dram_tensor("ps", [BPC, P, H], F32, kind="ExternalInput")
    cs = nc.dram_tensor("cs", [BPC, C, H], F32, kind="ExternalInput")
    w_ih_c = nc.dram_tensor("w_ih_c", [G, H], F32, kind="ExternalInput")
    w_hh_c = nc.dram_tensor("w_hh_c", [G, H], F32, kind="ExternalInput")
    w_ih_p = nc.dram_tensor("w_ih_p", [G, H], F32, kind="ExternalInput")
    w_hh_p = nc.dram_tensor("w_hh_p", [G, H], F32, kind="ExternalInput")
    b_ih_c = nc.dram_tensor("b_ih_c", [G, 1], F32, kind="ExternalInput")
    b_hh_c = nc.dram_tensor("b_hh_c", [G, 1], F32, kind="ExternalInput")
    b_ih_p = nc.dram_tensor("b_ih_p", [G, 1], F32, kind="ExternalInput")
    b_hh_p = nc.dram_tensor("b_hh_p", [G, 1], F32, kind="ExternalInput")
    out_np = nc.dram_tensor("new_path", [BPC, P, H], F32, kind="ExternalOutput")
    out_nc = nc.dram_tensor("new_channel", [BPC, C, H], F32, kind="ExternalOutput")
    dbg = {}
    if debug_outputs:
        dbg["cmT"] = nc.dram_tensor("dbg_cmT", [BPC, H, C], F32, kind="ExternalOutput")
        dbg["pmT"] = nc.dram_tensor("dbg_pmT", [BPC, H, P], F32, kind="ExternalOutput")
        dbg["ncsT"] = nc.dram_tensor("dbg_ncsT", [BPC, H, C], F32, kind="ExternalOutput")

    with tile.TileContext(nc) as tc:
        _body(tc, adj, ps, cs,
              (w_ih_c, w_hh_c, b_ih_c, b_hh_c),
              (w_ih_p, w_hh_p, b_ih_p, b_hh_p),
              out_np, out_nc, dbg)
    nc.finalize()
    return nc


def _body(tc, adj, ps, cs, wc, wp, out_np, out_nc, dbg):
    nc = tc.nc
    from contextlib import ExitStack

    ctx = ExitStack()
    with ctx:
        const = ctx.enter_context(tc.tile_pool(name="const", bufs=1))
        a_pool = ctx.enter_context(tc.tile_pool(name="a_slabs", bufs=4))
        at_pool = ctx.enter_context(tc.tile_pool(name="at", bufs=2))
        st_pool = ctx.enter_context(tc.tile_pool(name="states", bufs=1))
        gru_pool = ctx.enter_context(tc.tile_pool(name="gru", bufs=1))
        out_pool = ctx.enter_context(tc.tile_pool(name="outs", bufs=1))
        # PSUM banks: ps_mm 4 + ps_tp 2 + ps_misc 2 = 8
        ps_mm = ctx.enter_context(tc.tile_pool(name="ps_mm", bufs=1, space="PSUM"))
        ps_tp = ctx.enter_context(tc.tile_pool(name="ps_tp", bufs=2, space="PSUM"))
        ps_misc = ctx.enter_context(tc.tile_pool(name="ps_misc", bufs=2, space="PSUM"))

        ident = const.tile([PB, PB], BF16)
        masks.make_identity(nc, ident[:])
        ident_f = const.tile([PB, PB], F32)
        masks.make_identity(nc, ident_f[:])
        idents = {BF16: ident, F32: ident_f}

        # ---- weights: load [G, H], transpose to [H, G] via identity matmul ----
        # hhp stays f32: it multiplies path_msg (~1e5 scale) where the
        # z-gate argument needs small absolute error.
        wT = {}
        for name, wdram, wdt in (("ihc", wc[0], BF16), ("hhc", wc[1], BF16),
                                 ("ihp", wp[0], BF16), ("hhp", wp[1], F32)):
            w_ld = const.tile([G, H], wdt, tag=f"w_{name}")
            nc.gpsimd.dma_start(w_ld[:], wdram[:, :])
            wt_ps = ps_misc.tile([H, G], F32, tag="sm")
            nc.tensor.matmul(wt_ps[:], w_ld[:], idents[wdt][0:G, 0:G],
                             start=True, stop=True)
            wt = const.tile([H, G], wdt, tag=f"wT_{name}")
            nc.scalar.copy(wt[:], wt_ps[:])
            wT[name] = wt

        # ---- biases ----
        # bias_n[64:96] = b_ih_n, bias_n[96:128] = b_hh_n  (partition-aligned
        # with the PSUM gate layout: rows 64:96 = i_n, 96:128 = h_n)
        bias = {}
        for s, (bih, bhh) in (("c", (wc[2], wc[3])), ("p", (wp[2], wp[3]))):
            bn = const.tile([PB, 1], F32, tag=f"bn_{s}")
            nc.sync.dma_start(bn[2 * H : G, :], bih[2 * H : G, :])
            nc.sync.dma_start(bn[G : G + H, :], bhh[2 * H : G, :])
            ihrz = const.tile([2 * H, 1], F32, tag=f"bi_{s}")
            nc.sync.dma_start(ihrz[:], bih[0 : 2 * H, :])
            hhrz = const.tile([2 * H, 1], F32, tag=f"bh_{s}")
            nc.sync.dma_start(hhrz[:], bhh[0 : 2 * H, :])
            brz = const.tile([2 * H, 1], F32, tag=f"brz_{s}")
            nc.vector.tensor_add(brz[:], ihrz[:], hhrz[:])
            bias[s] = (brz, bn)

        for b in range(BPC):
            # ---- states: natural tiles (cast-DMA) + feature-major via PE ----
            ps_nat = st_pool.tile([PB, NP, H], BF16, tag="ps_nat")
            nc.gpsimd.dma_start(
                ps_nat[:], ps[b].rearrange("(i p) h -> p i h", p=PB))
            cs_nat = st_pool.tile([PB, NC, H], BF16, tag="cs_nat")
            nc.gpsimd.dma_start(
                cs_nat[:], cs[b].rearrange("(i p) h -> p i h", p=PB))

            sT = {}
            for nm, nat, nch in (("psT", ps_nat, NP), ("csT", cs_nat, NC)):
                dst = st_pool.tile([H, nch * PB], BF16, tag=nm)
                for quad in range(nch // 4):
                    tp = ps_misc.tile([H, 4, PB], F32, tag="sm")
                    for k in range(4):
                        nc.tensor.matmul(tp[:, k, :], nat[:, quad * 4 + k, :],
                                         ident[:, :], start=True, stop=True)
                    nc.scalar.copy(
                        dst[:, quad * 4 * PB : (quad + 1) * 4 * PB], tp[:])
                sT[nm] = dst

            # ---- stream A: einsum1 (cmT) + transposes into AT ----
            # transpose = regular identity matmul (out = slab_tile.T @ I):
            # pipelines at ~81ns and keeps the PE HAM clock warm, unlike
            # transpose-mode.
            at = at_pool.tile([PB, NC, P], BF16, tag="at")
            cmT = ps_mm.tile([H, C], F32, tag="mm")
            for i in range(NP):
                slab = a_pool.tile([PB, C], BF16, tag="a")
                nc.gpsimd.dma_start(slab[:], adj[b, i * PB : (i + 1) * PB, :])
                for n in range(C // NKC):
                    nc.tensor.matmul(
                        cmT[:, n * NKC : (n + 1) * NKC],
                        ps_nat[:, i, :],
                        slab[:, n * NKC : (n + 1) * NKC],
                        start=(i == 0), stop=(i == NP - 1),
                    )
                for quad in range(NC // 4):
                    tp = ps_tp.tile([PB, 4, PB], F32, tag="tp")
                    for k in range(4):
                        j = quad * 4 + k
                        nc.tensor.matmul(tp[:, k, :],
                                         slab[:, j * PB : (j + 1) * PB],
                                         ident[:, :], start=True, stop=True)
                    ev = at[:, quad * 4 : (quad + 1) * 4, i * PB : (i + 1) * PB]
                    if (i + quad) % 2 == 0:
                        nc.scalar.copy(ev, tp[:])
                    else:
                        nc.vector.tensor_copy(ev, tp[:])

            # ---- GRU-c ----
            cmT_s = st_pool.tile([H, C], BF16, tag="hback")
            nc.scalar.copy(cmT_s[:], cmT[:])
            if "cmT" in dbg:
                nc.gpsimd.dma_start(dbg["cmT"][b], cmT_s[:])

            ncsT = _gru(tc, gru_pool, ps_misc, wT["ihc"], wT["hhc"],
                        sT["csT"], cmT_s, bias["c"][0], bias["c"][1],
                        st_pool, "mid", dt_b=BF16, g_engine="act")

            if "ncsT" in dbg:
                nc.gpsimd.dma_start(dbg["ncsT"][b], ncsT[:])

            # ncs natural tiles [c_lo, j, H] for einsum2 lhsT
            ncs_nat = st_pool.tile([PB, NC, H], BF16, tag="ncs_nat")
            tpn = ps_misc.tile([PB, NC, H], F32, tag="sm")
            for j in range(NC):
                nc.tensor.matmul(tpn[:, j, :], ncsT[:, j * PB : (j + 1) * PB],
                                 ident[0:H, 0:H], start=True, stop=True)
            nc.vector.tensor_copy(ncs_nat[:], tpn[:])

            # packed output new_channel
            _pack_out(tc, ncsT, out_nc[b], ps_misc, out_pool, idents)

            # ---- einsum2: pmT ----
            pmT = ps_mm.tile([H, P], F32, tag="mm")
            for j in range(NC):
                for n in range(P // NKC):
                    nc.tensor.matmul(
                        pmT[:, n * NKC : (n + 1) * NKC],
                        ncs_nat[:, j, :],
                        at[:, j, n * NKC : (n + 1) * NKC],
                        start=(j == 0), stop=(j == NC - 1),
                    )
            pmT_s = st_pool.tile([H, P], F32, tag="mid")
            nc.scalar.copy(pmT_s[:], pmT[:])
            if "pmT" in dbg:
                nc.sync.dma_start(dbg["pmT"][b], pmT_s[:])

            # ---- GRU-p (f32 h-side: path_msg ~1e5 needs f32) ----
            npT = _gru(tc, gru_pool, ps_misc, wT["ihp"], wT["hhp"],
                       sT["psT"], pmT_s, bias["p"][0], bias["p"][1],
                       st_pool, "hback", dt_b=F32, g_engine="dve")

            _pack_out(tc, npT, out_np[b], ps_misc, out_pool, idents)


def _pack_out(tc, srcT, dram_b, ps_misc, out_pool, idents):
    """srcT [H, N] -> HBM [N, H] f32 with 512B-per-partition runs.

    Packed SBUF layout [q, g, l, h]: row index n = 512*g + 4*q + l.
    """
    nc = tc.nc
    dt = srcT.dtype
    N = srcT.shape[-1]
    NG = N // 512
    src_r = srcT.rearrange("h (g q l) -> h g q l", g=NG, l=4)
    sb = out_pool.tile([PB, NG, 4, H], F32, tag="opack")
    for g in range(NG):
        pk = ps_misc.tile([PB, 4, H], F32, tag="sm")
        for l in range(4):
            nc.tensor.matmul(pk[:, l, :], src_r[:, g, :, l],
                             idents[dt][0:H, 0:H], start=True, stop=True)
        nc.scalar.copy(sb[:, g, :, :], pk[:])
    nc.sync.dma_start(
        dram_b.rearrange("(g q l) h -> q g l h", q=PB, l=4), sb[:])


# ---------------------------------------------------------------------------
# host-side entry
# ---------------------------------------------------------------------------

_NC_CACHE = {}


def _get_nc(debug_outputs=False):
    key = bool(debug_outputs)
    if key not in _NC_CACHE:
        _NC_CACHE[key] = build_nc(debug_outputs=key)
    return _NC_CACHE[key]


def kernel(path_states, channel_states, adj_matrix,
           w_ih_c, w_hh_c, b_ih_c, b_hh_c,
           w_ih_p, w_hh_p, b_ih_p, b_hh_p,
           _debug=False, _trace=False):
    nc = _get_nc(debug_outputs=_debug)
    f32 = np.float32
    in_maps = []
    for k in range(NCORES):
        s = slice(k * BPC, (k + 1) * BPC)
        in_maps.append({
            "adj": np.ascontiguousarray(adj_matrix[s], f32),
            "ps": np.ascontiguousarray(path_states[s], f32),
            "cs": np.ascontiguousarray(channel_states[s], f32),
            "w_ih_c": np.ascontiguousarray(w_ih_c, f32),
            "w_hh_c": np.ascontiguousarray(w_hh_c, f32),
            "w_ih_p": np.ascontiguousarray(w_ih_p, f32),
            "w_hh_p": np.ascontiguousarray(w_hh_p, f32),
            "b_ih_c": np.ascontiguousarray(b_ih_c, f32).reshape(G, 1),
            "b_hh_c": np.ascontiguousarray(b_hh_c, f32).reshape(G, 1),
            "b_ih_p": np.ascontiguousarray(b_ih_p, f32).reshape(G, 1),
            "b_hh_p": np.ascontiguousarray(b_hh_p, f32).reshape(G, 1),
        })
    res = run_bass_kernel_spmd(nc, in_maps, core_ids=list(range(NCORES)),
                               trace=_trace)
    new_path = np.concatenate([res.results[k]["new_path"] for k in range(NCORES)])
    new_channel = np.concatenate(
        [res.results[k]["new_channel"] for k in range(NCORES)])
    out = (new_path, new_channel)
    if _debug or _trace:
        return out, res
    return out


# revision 23
# speedup vs baseline: 1.1778x; 1.1165x over previous
"""Trainium2 Bass kernel for nn_MessagePassingLayer (GNN message passing).

reference semantics (per batch b):
  cm  = adj[b].T @ ps[b]                  # [C, H] channel aggregation
  ncs = GRUCell(x=cs[b], h=cm)            # new channel states
  pm  = adj[b] @ ncs                      # [P, H] path aggregation
  nps = GRUCell(x=ps[b], h=pm)            # new path states
  returns (nps, ncs)

Sharding: data-parallel over batch, 2 batches per core x 8 cores.

Per-core design (memory-regime: adj is 16MB/batch and is the traffic):
  - adj[b] is DMA'd from HBM ONCE (f32->bf16 cast in flight, SWDGE),
    consumed as p-slabs [128, C] by einsum1 (moving operand), and
    PE-transposed tile-by-tile into a persistent AT [c_lo, j, p] bf16
    for einsum2. The reference reads adj twice; we read once.
  - einsum1 computed transposed: cmT[h, c] += ps_tile.T-form matmuls
    (lhsT = ps tile [p,H], rhs = A slab) accumulating in PSUM f32.
  - GRU gates feature-major: giT/ghT [3H, n] = w^T-form matmuls with
    rhs = xT/hT [H, n]; biases are per-partition ACT bias APs.
  - einsum2 transposed: pmT[h, p] += (lhsT = ncs tile [c,H],
    rhs = AT slab [c, p]) accumulating in PSUM f32.
  - outputs packed on-chip to [q, (g l h)] so each partition's HBM run
    is 512B (DMA line-rate), via stride-4 PE transposes.
"""

import numpy as np

import concourse.bass as bass
import concourse.tile as tile
from concourse import bacc, masks, mybir
from concourse.bass_utils import run_bass_kernel_spmd

F32 = mybir.dt.float32
# 2-byte compute dtype: fp16 (10-bit mantissa) — adj in [0,1), states O(1),
# messages O(30): all comfortably in fp16 range, 4x less rounding than bf16.
BF16 = mybir.dt.float16

B, P, C, H = 16, 2048, 2048, 32
G = 3 * H  # 96
NCORES = 8
BPC = B // NCORES  # batches per core
PB = 128  # partition block
NP = P // PB  # 16 p-chunks
NC = C // PB  # 16 c-chunks
NKC = 512  # matmul moving chunk (one PSUM f32 bank)


def _gru(tc, pool, ps_misc, wT_ih, wT_hh, xT, hT, b_rz, bias_n,
         st_pool, out_tag, dt_b=BF16, g_engine="act"):
    """Feature-major GRUCell -> SBUF [H, N] tile (dtype dt).

    Per 512-col chunk, one PSUM tile [128, 512] f32 holds:
      rows 0:64   = i_rz + h_rz   (two accumulating matmuls)
      rows 64:96  = i_n
      rows 96:128 = h_n
    r/z/gin/g are produced by single-input ACT/DVE ops reading PSUM
    (bias folded, base partition moved to 0), then 5 full-width
    TensorTensor ops combine. No gates SBUF tile, no separate adds.
    """
    nc = tc.nc
    AF = mybir.ActivationFunctionType
    N = xT.shape[-1]
    out = st_pool.tile([H, N], dt_b, tag=out_tag)
    r = pool.tile([H, N], BF16, tag="gru_r")
    z = pool.tile([H, N], BF16, tag="gru_z")
    g = pool.tile([H, N], BF16, tag="gru_g")
    gin = pool.tile([H, N], BF16, tag="gru_gin")
    for q in range(N // NKC):
        gp = ps_misc.tile([PB, NKC], F32, tag="sm")
        sl = slice(q * NKC, (q + 1) * NKC)
        nc.tensor.matmul(gp[0 : 2 * H, :], wT_ih[:, 0 : 2 * H], xT[:, sl],
                         start=True, stop=False)
        nc.tensor.matmul(gp[0 : 2 * H, :], wT_hh[:, 0 : 2 * H], hT[:, sl],
                         start=False, stop=True)
        nc.tensor.matmul(gp[2 * H : G, :], wT_ih[:, 2 * H : G], xT[:, sl],
                         start=True, stop=True)
        nc.tensor.matmul(gp[G : G + H, :], wT_hh[:, 2 * H : G], hT[:, sl],
                         start=True, stop=True, tile_position=(0, 96))
        nc.scalar.activation(r[:, sl], gp[0:H, :], AF.Sigmoid, bias=b_rz[0:H, :])
        nc.scalar.activation(z[:, sl], gp[H : 2 * H, :], AF.Sigmoid,
                             bias=b_rz[H : 2 * H, :])
        if g_engine == "act":
            nc.scalar.activation(g[:, sl], gp[G : G + H, :], AF.Identity,
                                 bias=bias_n[G : G + H, :])
        else:
            nc.vector.tensor_scalar_add(g[:, sl], gp[G : G + H, :],
                                        bias_n[G : G + H, :])
        nc.vector.tensor_scalar_add(gin[:, sl], gp[2 * H : G, :],
                                    bias_n[2 * H : G, :])
    t1 = pool.tile([H, N], BF16, tag="gru_t1")
    nc.vector.tensor_mul(t1[:], r[:], g[:])
    npre = pool.tile([H, N], BF16, tag="gru_g")
    nc.vector.tensor_add(npre[:], gin[:], t1[:])
    ng = pool.tile([H, N], dt_b, tag="gru_t1")
    nc.scalar.activation(ng[:], npre[:], AF.Tanh)
    d = pool.tile([H, N], dt_b, tag="gru_g")
    nc.vector.tensor_sub(d[:], hT, ng[:])
    zd = pool.tile([H, N], dt_b, tag="gru_gin")
    nc.vector.tensor_mul(zd[:], z[:], d[:])
    nc.vector.tensor_add(out[:], ng[:], zd[:])
    return out


def build_nc(debug_outputs=False):
    nc = bacc.Bacc("TRN2", target_bir_lowering=False, debug=False,
                   num_devices=NCORES)

    adj = nc.dram_tensor("adj", [BPC, P, C], F32, kind="ExternalInput")
    ps = nc.dram_tensor("ps", [BPC, P, H], F32, kind="ExternalInput")
    cs = nc.dram_tensor("cs", [BPC, C, H], F32, kind="ExternalInput")
    w_ih_c = nc.dram_tensor("w_ih_c", [G, H], F32, kind="ExternalInput")
    w_hh_c = nc.dram_tensor("w_hh_c", [G, H], F32, kind="ExternalInput")
    w_ih_p = nc.dram_tensor("w_ih_p", [G, H], F32, kind="ExternalInput")
    w_hh_p = nc.dram_tensor("w_hh_p", [G, H], F32, kind="ExternalInput")
    b_ih_c = nc.dram_tensor("b_ih_c", [G, 1], F32, kind="ExternalInput")
    b_hh_c = nc.dram_tensor("b_hh_c", [G, 1], F32, kind="ExternalInput")
    b_ih_p = nc.dram_tensor("b_ih_p", [G, 1], F32, kind="ExternalInput")
    b_hh_p = nc.dram_tensor("b_hh_p", [G, 1], F32, kind="ExternalInput")
    out_np = nc.dram_tensor("new_path", [BPC, P, H], F32, kind="ExternalOutput")
    out_nc = nc.dram_tensor("new_channel", [BPC, C, H], F32, kind="ExternalOutput")
    dbg = {}
    if debug_outputs:
        dbg["cmT"] = nc.dram_tensor("dbg_cmT", [BPC, H, C], F32, kind="ExternalOutput")
        dbg["pmT"] = nc.dram_tensor("dbg_pmT", [BPC, H, P], F32, kind="ExternalOutput")
        dbg["ncsT"] = nc.dram_tensor("dbg_ncsT", [BPC, H, C], F32, kind="ExternalOutput")

    with tile.TileContext(nc) as tc:
        _body(tc, adj, ps, cs,
              (w_ih_c, w_hh_c, b_ih_c, b_hh_c),
              (w_ih_p, w_hh_p, b_ih_p, b_hh_p),
              out_np, out_nc, dbg)
    nc.finalize()
    return nc


def _body(tc, adj, ps, cs, wc, wp, out_np, out_nc, dbg):
    nc = tc.nc
    from contextlib import ExitStack

    ctx = ExitStack()
    with ctx:
        const = ctx.enter_context(tc.tile_pool(name="const", bufs=1))
        a_pool = ctx.enter_context(tc.tile_pool(name="a_slabs", bufs=4))
        at_pool = ctx.enter_context(tc.tile_pool(name="at", bufs=2))
        st_pool = ctx.enter_context(tc.tile_pool(name="states", bufs=1))
        gru_pool = ctx.enter_context(tc.tile_pool(name="gru", bufs=1))
        out_pool = ctx.enter_context(tc.tile_pool(name="outs", bufs=1))
        # PSUM banks: ps_mm 4 + ps_tp 2 + ps_misc 2 = 8
        ps_mm = ctx.enter_context(tc.tile_pool(name="ps_mm", bufs=1, space="PSUM"))
        ps_tp = ctx.enter_context(tc.tile_pool(name="ps_tp", bufs=2, space="PSUM"))
        ps_misc = ctx.enter_context(tc.tile_pool(name="ps_misc", bufs=2, space="PSUM"))

        ident = const.tile([PB, PB], BF16)
        masks.make_identity(nc, ident[:])
        ident_f = const.tile([PB, PB], F32)
        masks.make_identity(nc, ident_f[:])
        idents = {BF16: ident, F32: ident_f}

        # ---- weights: load [G, H], transpose to [H, G] via identity matmul ----
        # hhp stays f32: it multiplies path_msg (~1e5 scale) where the
        # z-gate argument needs small absolute error.
        wT = {}
        for name, wdram, wdt in (("ihc", wc[0], BF16), ("hhc", wc[1], BF16),
                                 ("ihp", wp[0], BF16), ("hhp", wp[1], F32)):
            w_ld = const.tile([G, H], wdt, tag=f"w_{name}")
            nc.gpsimd.dma_start(w_ld[:], wdram[:, :])
            wt_ps = ps_misc.tile([H, G], F32, tag="sm")
            nc.tensor.matmul(wt_ps[:], w_ld[:], idents[wdt][0:G, 0:G],
                             start=True, stop=True)
            wt = const.tile([H, G], wdt, tag=f"wT_{name}")
            nc.scalar.copy(wt[:], wt_ps[:])
            wT[name] = wt

        # ---- biases ----
        # bias_n[64:96] = b_ih_n, bias_n[96:128] = b_hh_n  (partition-aligned
        # with the PSUM gate layout: rows 64:96 = i_n, 96:128 = h_n)
        bias = {}
        for s, (bih, bhh) in (("c", (wc[2], wc[3])), ("p", (wp[2], wp[3]))):
            bn = const.tile([PB, 1], F32, tag=f"bn_{s}")
            nc.sync.dma_start(bn[2 * H : G, :], bih[2 * H : G, :])
            nc.sync.dma_start(bn[G : G + H, :], bhh[2 * H : G, :])
            ihrz = const.tile([2 * H, 1], F32, tag=f"bi_{s}")
            nc.sync.dma_start(ihrz[:], bih[0 : 2 * H, :])
            hhrz = const.tile([2 * H, 1], F32, tag=f"bh_{s}")
            nc.sync.dma_start(hhrz[:], bhh[0 : 2 * H, :])
            brz = const.tile([2 * H, 1], F32, tag=f"brz_{s}")
            nc.vector.tensor_add(brz[:], ihrz[:], hhrz[:])
            bias[s] = (brz, bn)

        for b in range(BPC):
            # ---- states: natural tiles (cast-DMA) + feature-major via PE ----
            ps_nat = st_pool.tile([PB, NP, H], BF16, tag="ps_nat")
            nc.gpsimd.dma_start(
                ps_nat[:], ps[b].rearrange("(i p) h -> p i h", p=PB))
            cs_nat = st_pool.tile([PB, NC, H], BF16, tag="cs_nat")
            nc.gpsimd.dma_start(
                cs_nat[:], cs[b].rearrange("(i p) h -> p i h", p=PB))

            sT = {}
            for nm, nat, nch in (("psT", ps_nat, NP), ("csT", cs_nat, NC)):
                dst = st_pool.tile([H, nch * PB], BF16, tag=nm)
                for quad in range(nch // 4):
                    tp = ps_misc.tile([H, 4, PB], F32, tag="sm")
                    for k in range(4):
                        nc.tensor.matmul(tp[:, k, :], nat[:, quad * 4 + k, :],
                                         ident[:, :], start=True, stop=True)
                    nc.scalar.copy(
                        dst[:, quad * 4 * PB : (quad + 1) * 4 * PB], tp[:])
                sT[nm] = dst

            # ---- stream A: einsum1 (cmT) + transposes into AT ----
            # transpose = regular identity matmul (out = slab_tile.T @ I):
            # pipelines at ~81ns and keeps the PE HAM clock warm, unlike
            # transpose-mode.
            at = at_pool.tile([PB, NC, P], BF16, tag="at")
            # col-packed 4x einsum: group g computes cm^T[:, 512g:512(g+1)]
            # on PE column-group g -> PSUM partitions 32g, bank g.
            cmT = ps_mm.tile([PB, 4, NKC], F32, tag="mm")
            for i in range(NP):
                slab = a_pool.tile([PB, C], BF16, tag="a")
                nc.gpsimd.dma_start(slab[:], adj[b, i * PB : (i + 1) * PB, :])
                for n in range(C // NKC):
                    nc.tensor.matmul(
                        cmT[n * H : (n + 1) * H, n, :],
                        ps_nat[:, i, :],
                        slab[:, n * NKC : (n + 1) * NKC],
                        start=(i == 0), stop=(i == NP - 1),
                        tile_position=(0, n * H),
                    )
                for quad in range(NC // 4):
                    tp = ps_tp.tile([PB, 4, PB], F32, tag="tp")
                    for k in range(4):
                        j = quad * 4 + k
                        nc.tensor.matmul(tp[:, k, :],
                                         slab[:, j * PB : (j + 1) * PB],
                                         ident[:, :], start=True, stop=True)
                    ev = at[:, quad * 4 : (quad + 1) * 4, i * PB : (i + 1) * PB]
                    if (i + quad) % 2 == 0:
                        nc.scalar.copy(ev, tp[:])
                    else:
                        nc.vector.tensor_copy(ev, tp[:])

            # ---- GRU-c ----
            cmT_s = st_pool.tile([H, C], BF16, tag="hback")
            for n in range(4):
                nc.scalar.copy(cmT_s[:, n * NKC : (n + 1) * NKC],
                               cmT[n * H : (n + 1) * H, n, :])
            if "cmT" in dbg:
                nc.gpsimd.dma_start(dbg["cmT"][b], cmT_s[:])

            ncsT = _gru(tc, gru_pool, ps_misc, wT["ihc"], wT["hhc"],
                        sT["csT"], cmT_s, bias["c"][0], bias["c"][1],
                        st_pool, "mid", dt_b=BF16, g_engine="act")

            if "ncsT" in dbg:
                nc.gpsimd.dma_start(dbg["ncsT"][b], ncsT[:])

            # ncs natural tiles [c_lo, j, H] for einsum2 lhsT
            ncs_nat = st_pool.tile([PB, NC, H], BF16, tag="ncs_nat")
            tpn = ps_misc.tile([PB, NC, H], F32, tag="sm")
            for j in range(NC):
                nc.tensor.matmul(tpn[:, j, :], ncsT[:, j * PB : (j + 1) * PB],
                                 ident[0:H, 0:H], start=True, stop=True)
            nc.vector.tensor_copy(ncs_nat[:], tpn[:])

            # packed output new_channel
            _pack_out(tc, ncsT, out_nc[b], ps_misc, out_pool, idents)

            # ---- einsum2: pmT ----
            pmT = ps_mm.tile([PB, 4, NKC], F32, tag="mm")
            for j in range(NC):
                for n in range(P // NKC):
                    nc.tensor.matmul(
                        pmT[n * H : (n + 1) * H, n, :],
                        ncs_nat[:, j, :],
                        at[:, j, n * NKC : (n + 1) * NKC],
                        start=(j == 0), stop=(j == NC - 1),
                        tile_position=(0, n * H),
                    )
            pmT_s = st_pool.tile([H, P], F32, tag="mid")
            for n in range(4):
                nc.scalar.copy(pmT_s[:, n * NKC : (n + 1) * NKC],
                               pmT[n * H : (n + 1) * H, n, :])
            if "pmT" in dbg:
                nc.sync.dma_start(dbg["pmT"][b], pmT_s[:])

            # ---- GRU-p (f32 h-side: path_msg ~1e5 needs f32) ----
            npT = _gru(tc, gru_pool, ps_misc, wT["ihp"], wT["hhp"],
                       sT["psT"], pmT_s, bias["p"][0], bias["p"][1],
                       st_pool, "hback", dt_b=F32, g_engine="dve")

            _pack_out(tc, npT, out_np[b], ps_misc, out_pool, idents)


def _pack_out(tc, srcT, dram_b, ps_misc, out_pool, idents):
    """srcT [H, N] -> HBM [N, H] f32 with 512B-per-partition runs.

    Packed SBUF layout [q, g, l, h]: row index n = 512*g + 4*q + l.
    """
    nc = tc.nc
    dt = srcT.dtype
    N = srcT.shape[-1]
    NG = N // 512
    src_r = srcT.rearrange("h (g q l) -> h g q l", g=NG, l=4)
    sb = out_pool.tile([PB, NG, 4, H], F32, tag="opack")
    for g in range(NG):
        pk = ps_misc.tile([PB, 4, H], F32, tag="sm")
        for l in range(4):
            nc.tensor.matmul(pk[:, l, :], src_r[:, g, :, l],
                             idents[dt][0:H, 0:H], start=True, stop=True)
        nc.scalar.copy(sb[:, g, :, :], pk[:])
    nc.sync.dma_start(
        dram_b.rearrange("(g q l) h -> q g l h", q=PB, l=4), sb[:])


# ---------------------------------------------------------------------------
# host-side entry
# ---------------------------------------------------------------------------

_NC_CACHE = {}


def _get_nc(debug_outputs=False):
    key = bool(debug_outputs)
    if key not in _NC_CACHE:
        _NC_CACHE[key] = build_nc(debug_outputs=key)
    return _NC_CACHE[key]


def kernel(path_states, channel_states, adj_matrix,
           w_ih_c, w_hh_c, b_ih_c, b_hh_c,
           w_ih_p, w_hh_p, b_ih_p, b_hh_p,
           _debug=False, _trace=False):
    nc = _get_nc(debug_outputs=_debug)
    f32 = np.float32
    in_maps = []
    for k in range(NCORES):
        s = slice(k * BPC, (k + 1) * BPC)
        in_maps.append({
            "adj": np.ascontiguousarray(adj_matrix[s], f32),
            "ps": np.ascontiguousarray(path_states[s], f32),
            "cs": np.ascontiguousarray(channel_states[s], f32),
            "w_ih_c": np.ascontiguousarray(w_ih_c, f32),
            "w_hh_c": np.ascontiguousarray(w_hh_c, f32),
            "w_ih_p": np.ascontiguousarray(w_ih_p, f32),
            "w_hh_p": np.ascontiguousarray(w_hh_p, f32),
            "b_ih_c": np.ascontiguousarray(b_ih_c, f32).reshape(G, 1),
            "b_hh_c": np.ascontiguousarray(b_hh_c, f32).reshape(G, 1),
            "b_ih_p": np.ascontiguousarray(b_ih_p, f32).reshape(G, 1),
            "b_hh_p": np.ascontiguousarray(b_hh_p, f32).reshape(G, 1),
        })
    res = run_bass_kernel_spmd(nc, in_maps, core_ids=list(range(NCORES)),
                               trace=_trace)
    new_path = np.concatenate([res.results[k]["new_path"] for k in range(NCORES)])
    new_channel = np.concatenate(
        [res.results[k]["new_channel"] for k in range(NCORES)])
    out = (new_path, new_channel)
    if _debug or _trace:
        return out, res
    return out


# revision 24
# speedup vs baseline: 1.1802x; 1.0020x over previous
"""Trainium2 Bass kernel for nn_MessagePassingLayer (GNN message passing).

reference semantics (per batch b):
  cm  = adj[b].T @ ps[b]                  # [C, H] channel aggregation
  ncs = GRUCell(x=cs[b], h=cm)            # new channel states
  pm  = adj[b] @ ncs                      # [P, H] path aggregation
  nps = GRUCell(x=ps[b], h=pm)            # new path states
  returns (nps, ncs)

Sharding: data-parallel over batch, 2 batches per core x 8 cores.

Per-core design (memory-regime: adj is 16MB/batch and is the traffic):
  - adj[b] is DMA'd from HBM ONCE (f32->bf16 cast in flight, SWDGE),
    consumed as p-slabs [128, C] by einsum1 (moving operand), and
    PE-transposed tile-by-tile into a persistent AT [c_lo, j, p] bf16
    for einsum2. The reference reads adj twice; we read once.
  - einsum1 computed transposed: cmT[h, c] += ps_tile.T-form matmuls
    (lhsT = ps tile [p,H], rhs = A slab) accumulating in PSUM f32.
  - GRU gates feature-major: giT/ghT [3H, n] = w^T-form matmuls with
    rhs = xT/hT [H, n]; biases are per-partition ACT bias APs.
  - einsum2 transposed: pmT[h, p] += (lhsT = ncs tile [c,H],
    rhs = AT slab [c, p]) accumulating in PSUM f32.
  - outputs packed on-chip to [q, (g l h)] so each partition's HBM run
    is 512B (DMA line-rate), via stride-4 PE transposes.
"""

import numpy as np

import concourse.bass as bass
import concourse.tile as tile
from concourse import bacc, masks, mybir
from concourse.bass_utils import run_bass_kernel_spmd

F32 = mybir.dt.float32
# 2-byte compute dtype: fp16 (10-bit mantissa) — adj in [0,1), states O(1),
# messages O(30): all comfortably in fp16 range, 4x less rounding than bf16.
BF16 = mybir.dt.float16

B, P, C, H = 16, 2048, 2048, 32
G = 3 * H  # 96
NCORES = 8
BPC = B // NCORES  # batches per core
PB = 128  # partition block
NP = P // PB  # 16 p-chunks
NC = C // PB  # 16 c-chunks
NKC = 512  # matmul moving chunk (one PSUM f32 bank)


def _gru(tc, pool, ps_misc, wT_ih, wT_hh, xT, hT, b_rz, bias_n,
         st_pool, out_tag, dt_b=BF16, g_engine="act"):
    """Feature-major GRUCell -> SBUF [H, N] tile (dtype dt).

    Per 512-col chunk, one PSUM tile [128, 512] f32 holds:
      rows 0:64   = i_rz + h_rz   (two accumulating matmuls)
      rows 64:96  = i_n
      rows 96:128 = h_n
    r/z/gin/g are produced by single-input ACT/DVE ops reading PSUM
    (bias folded, base partition moved to 0), then 5 full-width
    TensorTensor ops combine. No gates SBUF tile, no separate adds.
    """
    nc = tc.nc
    AF = mybir.ActivationFunctionType
    N = xT.shape[-1]
    out = st_pool.tile([H, N], dt_b, tag=out_tag)
    r = pool.tile([H, N], BF16, tag="gru_r")
    z = pool.tile([H, N], BF16, tag="gru_z")
    g = pool.tile([H, N], BF16, tag="gru_g")
    gin = pool.tile([H, N], BF16, tag="gru_gin")
    for q in range(N // NKC):
        gp = ps_misc.tile([PB, NKC], F32, tag="sm")
        sl = slice(q * NKC, (q + 1) * NKC)
        nc.tensor.matmul(gp[0 : 2 * H, :], wT_ih[:, 0 : 2 * H], xT[:, sl],
                         start=True, stop=False)
        nc.tensor.matmul(gp[0 : 2 * H, :], wT_hh[:, 0 : 2 * H], hT[:, sl],
                         start=False, stop=True)
        nc.tensor.matmul(gp[2 * H : G, :], wT_ih[:, 2 * H : G], xT[:, sl],
                         start=True, stop=True)
        nc.tensor.matmul(gp[G : G + H, :], wT_hh[:, 2 * H : G], hT[:, sl],
                         start=True, stop=True, tile_position=(0, 96))
        nc.scalar.activation(r[:, sl], gp[0:H, :], AF.Sigmoid, bias=b_rz[0:H, :])
        nc.scalar.activation(z[:, sl], gp[H : 2 * H, :], AF.Sigmoid,
                             bias=b_rz[H : 2 * H, :])
        if g_engine == "act":
            nc.scalar.activation(g[:, sl], gp[G : G + H, :], AF.Identity,
                                 bias=bias_n[G : G + H, :])
        else:
            nc.vector.tensor_scalar_add(g[:, sl], gp[G : G + H, :],
                                        bias_n[G : G + H, :])
        nc.vector.tensor_scalar_add(gin[:, sl], gp[2 * H : G, :],
                                    bias_n[2 * H : G, :])
    t1 = pool.tile([H, N], BF16, tag="gru_t1")
    nc.vector.tensor_mul(t1[:], r[:], g[:])
    npre = pool.tile([H, N], BF16, tag="gru_g")
    nc.vector.tensor_add(npre[:], gin[:], t1[:])
    ng = pool.tile([H, N], dt_b, tag="gru_t1")
    nc.scalar.activation(ng[:], npre[:], AF.Tanh)
    d = pool.tile([H, N], dt_b, tag="gru_g")
    nc.vector.tensor_sub(d[:], hT, ng[:])
    zd = pool.tile([H, N], dt_b, tag="gru_gin")
    nc.vector.tensor_mul(zd[:], z[:], d[:])
    nc.vector.tensor_add(out[:], ng[:], zd[:])
    return out


def build_nc(debug_outputs=False):
    nc = bacc.Bacc("TRN2", target_bir_lowering=False, debug=False,
                   num_devices=NCORES)

    adj = nc.dram_tensor("adj", [BPC, P, C], F32, kind="ExternalInput")
    ps = nc.dram_tensor("ps", [BPC, P, H], F32, kind="ExternalInput")
    cs = nc.dram_tensor("cs", [BPC, C, H], F32, kind="ExternalInput")
    w_ih_c = nc.dram_tensor("w_ih_c", [G, H], F32, kind="ExternalInput")
    w_hh_c = nc.dram_tensor("w_hh_c", [G, H], F32, kind="ExternalInput")
    w_ih_p = nc.dram_tensor("w_ih_p", [G, H], F32, kind="ExternalInput")
    w_hh_p = nc.dram_tensor("w_hh_p", [G, H], F32, kind="ExternalInput")
    b_ih_c = nc.dram_tensor("b_ih_c", [G, 1], F32, kind="ExternalInput")
    b_hh_c = nc.dram_tensor("b_hh_c", [G, 1], F32, kind="ExternalInput")
    b_ih_p = nc.dram_tensor("b_ih_p", [G, 1], F32, kind="ExternalInput")
    b_hh_p = nc.dram_tensor("b_hh_p", [G, 1], F32, kind="ExternalInput")
    out_np = nc.dram_tensor("new_path", [BPC, P, H], F32, kind="ExternalOutput")
    out_nc = nc.dram_tensor("new_channel", [BPC, C, H], F32, kind="ExternalOutput")
    dbg = {}
    if debug_outputs:
        dbg["cmT"] = nc.dram_tensor("dbg_cmT", [BPC, H, C], F32, kind="ExternalOutput")
        dbg["pmT"] = nc.dram_tensor("dbg_pmT", [BPC, H, P], F32, kind="ExternalOutput")
        dbg["ncsT"] = nc.dram_tensor("dbg_ncsT", [BPC, H, C], F32, kind="ExternalOutput")

    with tile.TileContext(nc) as tc:
        _body(tc, adj, ps, cs,
              (w_ih_c, w_hh_c, b_ih_c, b_hh_c),
              (w_ih_p, w_hh_p, b_ih_p, b_hh_p),
              out_np, out_nc, dbg)
    nc.finalize()
    return nc


def _body(tc, adj, ps, cs, wc, wp, out_np, out_nc, dbg):
    nc = tc.nc
    from contextlib import ExitStack

    ctx = ExitStack()
    with ctx:
        const = ctx.enter_context(tc.tile_pool(name="const", bufs=1))
        a_pool = ctx.enter_context(tc.tile_pool(name="a_slabs", bufs=4))
        at_pool = ctx.enter_context(tc.tile_pool(name="at", bufs=2))
        st_pool = ctx.enter_context(tc.tile_pool(name="states", bufs=1))
        gru_pool = ctx.enter_context(tc.tile_pool(name="gru", bufs=1))
        out_pool = ctx.enter_context(tc.tile_pool(name="outs", bufs=1))
        # PSUM banks: ps_mm 4 + ps_tp 2 + ps_misc 2 = 8
        ps_mm = ctx.enter_context(tc.tile_pool(name="ps_mm", bufs=1, space="PSUM"))
        ps_tp = ctx.enter_context(tc.tile_pool(name="ps_tp", bufs=2, space="PSUM"))
        ps_misc = ctx.enter_context(tc.tile_pool(name="ps_misc", bufs=2, space="PSUM"))

        ident = const.tile([PB, PB], BF16)
        masks.make_identity(nc, ident[:])
        ident_f = const.tile([PB, PB], F32)
        masks.make_identity(nc, ident_f[:])
        idents = {BF16: ident, F32: ident_f}

        # ---- weights: load [G, H], transpose to [H, G] via identity matmul ----
        # hhp stays f32: it multiplies path_msg (~1e5 scale) where the
        # z-gate argument needs small absolute error.
        wT = {}
        for name, wdram, wdt in (("ihc", wc[0], BF16), ("hhc", wc[1], BF16),
                                 ("ihp", wp[0], BF16), ("hhp", wp[1], F32)):
            w_ld = const.tile([G, H], wdt, tag=f"w_{name}")
            nc.gpsimd.dma_start(w_ld[:], wdram[:, :])
            wt_ps = ps_misc.tile([H, G], F32, tag="sm")
            nc.tensor.matmul(wt_ps[:], w_ld[:], idents[wdt][0:G, 0:G],
                             start=True, stop=True)
            wt = const.tile([H, G], wdt, tag=f"wT_{name}")
            nc.scalar.copy(wt[:], wt_ps[:])
            wT[name] = wt

        # ---- biases ----
        # bias_n[64:96] = b_ih_n, bias_n[96:128] = b_hh_n  (partition-aligned
        # with the PSUM gate layout: rows 64:96 = i_n, 96:128 = h_n)
        bias = {}
        for s, (bih, bhh) in (("c", (wc[2], wc[3])), ("p", (wp[2], wp[3]))):
            bn = const.tile([PB, 1], F32, tag=f"bn_{s}")
            nc.sync.dma_start(bn[2 * H : G, :], bih[2 * H : G, :])
            nc.sync.dma_start(bn[G : G + H, :], bhh[2 * H : G, :])
            ihrz = const.tile([2 * H, 1], F32, tag=f"bi_{s}")
            nc.sync.dma_start(ihrz[:], bih[0 : 2 * H, :])
            hhrz = const.tile([2 * H, 1], F32, tag=f"bh_{s}")
            nc.sync.dma_start(hhrz[:], bhh[0 : 2 * H, :])
            brz = const.tile([2 * H, 1], F32, tag=f"brz_{s}")
            nc.vector.tensor_add(brz[:], ihrz[:], hhrz[:])
            bias[s] = (brz, bn)

        for b in range(BPC):
            # ---- states: natural tiles (cast-DMA) + feature-major via PE ----
            ps_nat = st_pool.tile([PB, NP, H], BF16, tag="ps_nat")
            nc.gpsimd.dma_start(
                ps_nat[:], ps[b].rearrange("(i p) h -> p i h", p=PB))
            cs_nat = st_pool.tile([PB, NC, H], BF16, tag="cs_nat")
            nc.gpsimd.dma_start(
                cs_nat[:], cs[b].rearrange("(i p) h -> p i h", p=PB))

            sT = {}
            for nm, nat, nch in (("psT", ps_nat, NP), ("csT", cs_nat, NC)):
                dst = st_pool.tile([H, nch * PB], BF16, tag=nm)
                for quad in range(nch // 4):
                    tp = ps_misc.tile([H, 4, PB], F32, tag="sm")
                    for k in range(4):
                        nc.tensor.matmul(tp[:, k, :], nat[:, quad * 4 + k, :],
                                         ident[:, :], start=True, stop=True)
                    nc.scalar.copy(
                        dst[:, quad * 4 * PB : (quad + 1) * 4 * PB], tp[:])
                sT[nm] = dst

            # ---- stream A: einsum1 (cmT) + transposes into AT ----
            # transpose = regular identity matmul (out = slab_tile.T @ I):
            # pipelines at ~81ns and keeps the PE HAM clock warm, unlike
            # transpose-mode.
            at = at_pool.tile([PB, NC, P], BF16, tag="at")
            # col-packed 4x einsum: group g computes cm^T[:, 512g:512(g+1)]
            # on PE column-group g -> PSUM partitions 32g, bank g.
            cmT = ps_mm.tile([PB, 4, NKC], F32, tag="mm")
            for i in range(NP):
                slab = a_pool.tile([PB, C], BF16, tag="a")
                nc.gpsimd.dma_start(slab[:], adj[b, i * PB : (i + 1) * PB, :])
                for n in range(C // NKC):
                    nc.tensor.matmul(
                        cmT[n * H : (n + 1) * H, n, :],
                        ps_nat[:, i, :],
                        slab[:, n * NKC : (n + 1) * NKC],
                        start=(i == 0), stop=(i == NP - 1),
                        tile_position=(0, n * H),
                    )
                for quad in range(NC // 4):
                    tp = ps_tp.tile([PB, 4, PB], F32, tag="tp")
                    for k in range(4):
                        j = quad * 4 + k
                        # transpose tile j as 4 col-strips: weights are
                        # [128, 32] (cheap ldweights), strips run
                        # concurrently on distinct PE column-groups and
                        # stack vertically into the transposed tile.
                        for s in range(4):
                            nc.tensor.matmul(
                                tp[s * H : (s + 1) * H, k, :],
                                slab[:, j * PB + s * H : j * PB + (s + 1) * H],
                                ident[:, :], start=True, stop=True,
                                tile_position=(0, s * H),
                            )
                    ev = at[:, quad * 4 : (quad + 1) * 4, i * PB : (i + 1) * PB]
                    if (i + quad) % 2 == 0:
                        nc.scalar.copy(ev, tp[:])
                    else:
                        nc.vector.tensor_copy(ev, tp[:])

            # ---- GRU-c ----
            cmT_s = st_pool.tile([H, C], BF16, tag="hback")
            for n in range(4):
                nc.scalar.copy(cmT_s[:, n * NKC : (n + 1) * NKC],
                               cmT[n * H : (n + 1) * H, n, :])
            if "cmT" in dbg:
                nc.gpsimd.dma_start(dbg["cmT"][b], cmT_s[:])

            ncsT = _gru(tc, gru_pool, ps_misc, wT["ihc"], wT["hhc"],
                        sT["csT"], cmT_s, bias["c"][0], bias["c"][1],
                        st_pool, "mid", dt_b=BF16, g_engine="act")

            if "ncsT" in dbg:
                nc.gpsimd.dma_start(dbg["ncsT"][b], ncsT[:])

            # ncs natural tiles [c_lo, j, H] for einsum2 lhsT
            ncs_nat = st_pool.tile([PB, NC, H], BF16, tag="ncs_nat")
            tpn = ps_misc.tile([PB, NC, H], F32, tag="sm")
            for j in range(NC):
                nc.tensor.matmul(tpn[:, j, :], ncsT[:, j * PB : (j + 1) * PB],
                                 ident[0:H, 0:H], start=True, stop=True)
            nc.vector.tensor_copy(ncs_nat[:], tpn[:])

            # packed output new_channel
            _pack_out(tc, ncsT, out_nc[b], ps_misc, out_pool, idents)

            # ---- einsum2: pmT ----
            pmT = ps_mm.tile([PB, 4, NKC], F32, tag="mm")
            for j in range(NC):
                for n in range(P // NKC):
                    nc.tensor.matmul(
                        pmT[n * H : (n + 1) * H, n, :],
                        ncs_nat[:, j, :],
                        at[:, j, n * NKC : (n + 1) * NKC],
                        start=(j == 0), stop=(j == NC - 1),
                        tile_position=(0, n * H),
                    )
            pmT_s = st_pool.tile([H, P], F32, tag="mid")
            for n in range(4):
                nc.scalar.copy(pmT_s[:, n * NKC : (n + 1) * NKC],
                               pmT[n * H : (n + 1) * H, n, :])
            if "pmT" in dbg:
                nc.sync.dma_start(dbg["pmT"][b], pmT_s[:])

            # ---- GRU-p (f32 h-side: path_msg ~1e5 needs f32) ----
            npT = _gru(tc, gru_pool, ps_misc, wT["ihp"], wT["hhp"],
                       sT["psT"], pmT_s, bias["p"][0], bias["p"][1],
                       st_pool, "hback", dt_b=F32, g_engine="dve")

            _pack_out(tc, npT, out_np[b], ps_misc, out_pool, idents)


def _pack_out(tc, srcT, dram_b, ps_misc, out_pool, idents):
    """srcT [H, N] -> HBM [N, H] f32 with 512B-per-partition runs.

    Packed SBUF layout [q, g, l, h]: row index n = 512*g + 4*q + l.
    """
    nc = tc.nc
    dt = srcT.dtype
    N = srcT.shape[-1]
    NG = N // 512
    src_r = srcT.rearrange("h (g q l) -> h g q l", g=NG, l=4)
    sb = out_pool.tile([PB, NG, 4, H], F32, tag="opack")
    for g in range(NG):
        pk = ps_misc.tile([PB, 4, H], F32, tag="sm")
        for l in range(4):
            nc.tensor.matmul(pk[:, l, :], src_r[:, g, :, l],
                             idents[dt][0:H, 0:H], start=True, stop=True)
        nc.scalar.copy(sb[:, g, :, :], pk[:])
    nc.sync.dma_start(
        dram_b.rearrange("(g q l) h -> q g l h", q=PB, l=4), sb[:])


# ---------------------------------------------------------------------------
# host-side entry
# ---------------------------------------------------------------------------

_NC_CACHE = {}


def _get_nc(debug_outputs=False):
    key = bool(debug_outputs)
    if key not in _NC_CACHE:
        _NC_CACHE[key] = build_nc(debug_outputs=key)
    return _NC_CACHE[key]


def kernel(path_states, channel_states, adj_matrix,
           w_ih_c, w_hh_c, b_ih_c, b_hh_c,
           w_ih_p, w_hh_p, b_ih_p, b_hh_p,
           _debug=False, _trace=False):
    nc = _get_nc(debug_outputs=_debug)
    f32 = np.float32
    in_maps = []
    for k in range(NCORES):
        s = slice(k * BPC, (k + 1) * BPC)
        in_maps.append({
            "adj": np.ascontiguousarray(adj_matrix[s], f32),
            "ps": np.ascontiguousarray(path_states[s], f32),
            "cs": np.ascontiguousarray(channel_states[s], f32),
            "w_ih_c": np.ascontiguousarray(w_ih_c, f32),
            "w_hh_c": np.ascontiguousarray(w_hh_c, f32),
            "w_ih_p": np.ascontiguousarray(w_ih_p, f32),
            "w_hh_p": np.ascontiguousarray(w_hh_p, f32),
            "b_ih_c": np.ascontiguousarray(b_ih_c, f32).reshape(G, 1),
            "b_hh_c": np.ascontiguousarray(b_hh_c, f32).reshape(G, 1),
            "b_ih_p": np.ascontiguousarray(b_ih_p, f32).reshape(G, 1),
            "b_hh_p": np.ascontiguousarray(b_hh_p, f32).reshape(G, 1),
        })
    res = run_bass_kernel_spmd(nc, in_maps, core_ids=list(range(NCORES)),
                               trace=_trace)
    new_path = np.concatenate([res.results[k]["new_path"] for k in range(NCORES)])
    new_channel = np.concatenate(
        [res.results[k]["new_channel"] for k in range(NCORES)])
    out = (new_path, new_channel)
    if _debug or _trace:
        return out, res
    return out
